# revision 1
# baseline (speedup 1.0000x reference)
"""HeatmapMSELoss Trainium2 kernel.

Computes mean((heatmaps_pred - heatmaps_gt)^2) where heatmaps_gt is an
isotropic 2D gaussian (sigma=1, peak 1) rendered at the projection of each
3D joint into each view.

Key identity: the gaussian separates, gt[h,w] = gy[h] * gx[w], so

  sum_hw (pred - gt)^2 = sum_hw pred^2 - 2 * gy^T (pred @ gx) + (sum gy^2)(sum gx^2)

The 142MB gt tensor is never materialized. Per (b,v,j) slice the device
computes sum(pred^2) (scalar-engine square + accumulate) and
m' = pred^T @ gy (one matmul, PSUM column), then a fused DVE
multiply+reduce against gx. The tiny 1D gaussians (2.2MB total) and the
final scalar combine are done on host in float64.

Sharding: data-parallel over batch, 4 batches per core across 8 cores.
"""

import numpy as np

import concourse.bacc as bacc
import concourse.bass as bass
import concourse.tile as tile
from concourse import mybir
from concourse.bass_utils import run_bass_kernel_spmd

B, V, J, H, W = 32, 4, 17, 128, 128
N_CORES = 8
B_LOC = B // N_CORES          # 4 batches per core
GROUPS = B_LOC * V            # 16 (b,v) groups per core
SLICES = GROUPS * J           # 272 slices per core

_CACHE = {}


GPB = 2                    # (b,v) groups per block
NBLK = GROUPS // GPB       # blocks per core
JB = GPB * J               # joints (slices) per block

# chunk sizes (in slices) over the 272 per-core slices: small chunks at the
# start (fast pipeline ramp: compute starts after a ~1us DMA, not ~3us) and
# at the end (short tail after the last DMA lands)
CHUNKS = [4, 4, 4, 5] + [17] * 14 + [9, 8]
assert sum(CHUNKS) == SLICES


def _build_nc(passes=1, chunks=None, load_bufs=6):
    # Bacc (not raw Bass): its finalize() runs the legalization passes that
    # split multi-wait instructions (matmul can carry at most 1 sync wait).
    nc = bacc.Bacc()
    f32 = mybir.dt.float32
    chunks = list(CHUNKS) if chunks is None else list(chunks)
    nck = len(chunks)
    maxck = max(chunks)

    pred = nc.declare_dram_parameter("pred", [SLICES, H, W], f32, isOutput=False)
    gyt = nc.declare_dram_parameter("gyt", [H, SLICES], f32, isOutput=False)
    gxt = nc.declare_dram_parameter("gxt", [W, SLICES], f32, isOutput=False)
    partials = nc.declare_dram_parameter("partials", [128, 2, nck], f32, isOutput=True)

    with tile.TileContext(nc) as tc:
        with (
            tc.tile_pool(name="consts", bufs=1) as consts,
            tc.tile_pool(name="loads", bufs=load_bufs) as loads,
            tc.tile_pool(name="sq", bufs=2) as sqpool,
            tc.tile_pool(name="prod", bufs=2) as prodpool,
            tc.tile_pool(name="psum", bufs=4, space="PSUM") as psumpool,
            tc.tile_pool(name="outs", bufs=1) as outs,
        ):
            # warm-up ACT so the Square table-set load (~2.7us) overlaps the
            # first pred DMA instead of stalling the first real ACT
            warm = consts.tile([128, 1], f32)
            nc.vector.memset(warm[:], 0.0)
            wsq = consts.tile([128, 1], f32)
            nc.scalar.activation(
                out=wsq[:], in_=warm[:], func=mybir.ActivationFunctionType.Square
            )

            gyt_t = consts.tile([H, SLICES], f32)
            nc.sync.dma_start(out=gyt_t[:], in_=gyt[:, :])
            gxt_t = consts.tile([W, SLICES], f32)
            nc.sync.dma_start(out=gxt_t[:], in_=gxt[:, :])

            outcols = outs.tile([128, 2, nck], f32)

            for _p in range(passes):
                s0 = 0
                for c, csz in enumerate(chunks):
                    t = loads.tile([H, maxck, W], f32, tag="loads")
                    nc.sync.dma_start(
                        out=t[:, :csz, :],
                        in_=pred[s0 : s0 + csz].rearrange("s h w -> h s w"),
                    )

                    # s1: per-partition sum of pred^2 over (s, w)
                    sq = sqpool.tile([H, maxck, W], f32, tag="sq")
                    nc.scalar.activation(
                        out=sq[:, :csz, :],
                        in_=t[:, :csz, :],
                        func=mybir.ActivationFunctionType.Square,
                        accum_out=outcols[:, 0, c : c + 1],
                    )

                    # s2: m'_s = pred_s^T @ gy_s per slice -> psum column
                    ps = psumpool.tile([128, maxck], f32, tag="psum")
                    for sj in range(csz):
                        s = s0 + sj
                        nc.tensor.matmul(
                            ps[:, sj : sj + 1],
                            t[:, sj, :],
                            gyt_t[:, s : s + 1],
                            start=True,
                            stop=True,
                        )
                    # dot with gx, then per-partition sum over slices
                    prod = prodpool.tile([128, maxck], f32, tag="prod")
                    nc.vector.tensor_mul(
                        prod[:, :csz], ps[:, :csz], gxt_t[:, s0 : s0 + csz]
                    )
                    nc.vector.reduce_sum(
                        outcols[:, 1, c : c + 1], prod[:, :csz],
                        axis=mybir.AxisListType.X,
                    )
                    s0 += csz

            nc.sync.dma_start(out=partials[:, :, :], in_=outcols[:])

    nc.finalize()  # Bacc: runs legalization (wait splitting) + regalloc
    return nc


def _gaussians(proj_mats_batch, joints_3d_gt_batch):
    """1D gaussians gy [B,V,J,H], gx [B,V,J,W] in float32 (reference math)."""
    joints = joints_3d_gt_batch.astype(np.float32)
    ones = np.ones(joints.shape[:-1] + (1,), dtype=np.float32)
    joints_h = np.concatenate([joints, ones], axis=-1)  # [B, J, 4]
    proj = np.einsum(
        "bvcd,bjd->bvjc", proj_mats_batch.astype(np.float32), joints_h
    ).astype(np.float32)  # [B, V, J, 3]
    joints_2d = proj[..., :2] / proj[..., 2:3]  # (x, y)
    xs = np.arange(W, dtype=np.float32)
    ys = np.arange(H, dtype=np.float32)
    dx2 = (xs - joints_2d[..., 0, None]) ** 2  # [B,V,J,W]
    dy2 = (ys - joints_2d[..., 1, None]) ** 2  # [B,V,J,H]
    gx = np.exp(-0.5 * dx2).astype(np.float32)
    gy = np.exp(-0.5 * dy2).astype(np.float32)
    return gy, gx


def kernel(heatmaps_pred, proj_mats_batch, joints_3d_gt_batch, joints_3d_valid_batch,
           _profile=None):
    heatmaps_pred = np.ascontiguousarray(np.asarray(heatmaps_pred, dtype=np.float32))
    gy, gx = _gaussians(np.asarray(proj_mats_batch), np.asarray(joints_3d_gt_batch))

    # s3 = sum over slices of (sum_h gy^2) * (sum_w gx^2), exact in f64
    s3 = float(
        ((gy.astype(np.float64) ** 2).sum(-1) * (gx.astype(np.float64) ** 2).sum(-1)).sum()
    )

    if "nc" not in _CACHE:
        _CACHE["nc"] = _build_nc()
    nc = _CACHE["nc"]

    in_maps = []
    for c in range(N_CORES):
        bsl = slice(B_LOC * c, B_LOC * (c + 1))
        # slice order: (b_local, v, j) -> s ; tiles are [H|W, SLICES]
        gyt = np.ascontiguousarray(gy[bsl].reshape(SLICES, H).T)
        gxt = np.ascontiguousarray(gx[bsl].reshape(SLICES, W).T)
        in_maps.append(
            {
                "pred": heatmaps_pred[bsl].reshape(SLICES, H, W),
                "gyt": gyt,
                "gxt": gxt,
            }
        )

    res = run_bass_kernel_spmd(nc, in_maps, core_ids=list(range(N_CORES)))
    if _profile is not None:
        _profile["result"] = res
        _profile["in_maps"] = in_maps

    s1 = 0.0
    s2 = 0.0
    for c in range(N_CORES):
        p = res.results[c]["partials"].astype(np.float64)
        s1 += p[:, 0, :].sum()
        s2 += p[:, 1, :].sum()

    total = s1 - 2.0 * s2 + s3
    return np.float32(total / (B * V * J * H * W))



# revision 19
# speedup vs baseline: 1.0570x; 1.0570x over previous
"""HeatmapMSELoss Trainium2 kernel.

Computes mean((heatmaps_pred - heatmaps_gt)^2) where heatmaps_gt is an
isotropic 2D gaussian (sigma=1, peak 1) rendered at the projection of each
3D joint into each view.

Key identity: the gaussian separates, gt[h,w] = gy[h] * gx[w], so

  sum_hw (pred - gt)^2 = sum_hw pred^2 - 2 * gy^T (pred @ gx) + (sum gy^2)(sum gx^2)

The 142MB gt tensor is never materialized. Per (b,v,j) slice the device
computes sum(pred^2) (scalar-engine square + accumulate, with a DVE lane for
part of the tail) and m' = pred^T @ gy (one matmul, PSUM column), then a
fused DVE multiply+reduce against gx. The 1D gaussians are generated ON
DEVICE from a tiny [2, 672] host constant (outer-product matmul for h - y,
then square + exp on DVE/ACT); the scalar combine runs on host in float64.

The per-chunk partials leave via a SWDGE kv-writeback whose descriptors are
prepared early on the idle Pool engine; an end-of-kernel trigger fires them,
skipping the HWDGE+DGE init latency on the tail critical path. Chunk sizes
taper at the end so the last chunks' compute (gated at DMA+900ns) doesn't
trail the final transfer.

Sharding: data-parallel over batch, 4 batches per core across 8 cores.
"""

import numpy as np

import concourse.bacc as bacc
import concourse.bass as bass
import concourse.tile as tile
from concourse import mybir
from concourse.bass_utils import run_bass_kernel_spmd

B, V, J, H, W = 32, 4, 17, 128, 128
SIGMA = 1.0
N_CORES = 8
B_LOC = B // N_CORES          # 4 batches per core
GROUPS = B_LOC * V            # 16 (b,v) groups per core
SLICES = GROUPS * J           # 272 slices per core

# stream plan: (chunk size in slices, engine lane for the sum-of-squares).
# Bulk 17-slice chunks on ACT (1-pass square+accum), tapered tail so the
# last chunks' compute latency (900ns DMA-sem gate + engine time) doesn't
# trail the final transfer; one DVE chunk near the end unloads ACT.
PLAN = [(17, "act")] * 13 + [(13, "act")] + [
    (10, "act"), (8, "act"), (8, "act"), (6, "act"), (3, "dve"), (3, "act")
]
assert sum(n for n, _ in PLAN) == SLICES
NCK = len(PLAN)

_CACHE = {}


def _build_nc(plan=None, load_bufs=6):
    nc = bacc.Bacc()
    f32 = mybir.dt.float32
    plan = list(PLAN) if plan is None else list(plan)
    chunks = [c for c, _ in plan]
    nck = len(chunks)
    maxck = max(chunks)
    NCN = 2 * nck

    pred = nc.declare_dram_parameter("pred", [SLICES, H, W], f32, isOutput=False)
    consts = nc.declare_dram_parameter(
        "consts", [2, 128 + 2 * SLICES], f32, isOutput=False
    )
    partials = nc.declare_dram_parameter(
        "partials", [1, 128, 1, NCN], f32, isOutput=True
    )

    with tile.TileContext(nc) as tc:
        with (
            tc.tile_pool(name="cpool", bufs=1) as cpool,
            tc.tile_pool(name="loads", bufs=load_bufs) as loads,
            tc.tile_pool(name="sq", bufs=2) as sqpool,
            tc.tile_pool(name="prod", bufs=2) as prodpool,
            tc.tile_pool(name="psum", bufs=4, space="PSUM") as psumpool,
            tc.tile_pool(name="gpsum", bufs=2, space="PSUM") as gpsumpool,
            tc.tile_pool(name="outs", bufs=1) as outs,
        ):
            # ACT table warm-up (Square+Exp share a table set); the load
            # overlaps the first pred DMA
            warm = cpool.tile([128, 1], f32)
            nc.vector.memset(warm[:], 0.0)
            wsq = cpool.tile([128, 1], f32)
            nc.scalar.activation(
                out=wsq[:], in_=warm[:], func=mybir.ActivationFunctionType.Square
            )
            nc.scalar.activation(
                out=wsq[:], in_=warm[:], func=mybir.ActivationFunctionType.Exp
            )

            outcols = outs.tile([128, 1, 1, NCN], f32)

            # chunk 0 pred DMA issued FIRST so the big stream starts asap
            c0 = chunks[0]
            t0 = loads.tile([H, maxck, W], f32, tag="loads")
            nc.sync.dma_start(
                out=t0[:, :c0, :],
                in_=pred[0:c0].rearrange("s h w -> h s w"),
            )

            # consts DMA + on-device gaussian generation:
            # gy[h,s] = exp(-0.5 (h - y_s)^2), gx likewise, via an
            # outer-product matmul (dy = h - y) then square (DVE) + exp (ACT)
            ct = cpool.tile([2, 128 + 2 * SLICES], f32)
            nc.sync.dma_start(out=ct[:], in_=consts[:, :])
            psy = gpsumpool.tile([128, SLICES], f32)
            nc.tensor.matmul(
                psy[:], ct[:, 0:128], ct[:, 128 : 128 + SLICES], start=True, stop=True
            )
            psx = gpsumpool.tile([128, SLICES], f32)
            nc.tensor.matmul(
                psx[:],
                ct[:, 0:128],
                ct[:, 128 + SLICES : 128 + 2 * SLICES],
                start=True,
                stop=True,
            )
            gsq = cpool.tile([128, 2 * SLICES], f32)
            nc.scalar.activation(
                out=gsq[:, 0:SLICES], in_=psy[:],
                func=mybir.ActivationFunctionType.Square,
            )
            nc.scalar.activation(
                out=gsq[:, SLICES:], in_=psx[:],
                func=mybir.ActivationFunctionType.Square,
            )
            gyx = cpool.tile([128, 2 * SLICES], f32)
            nc.scalar.activation(
                out=gyx[:],
                in_=gsq[:],
                func=mybir.ActivationFunctionType.Exp,
                scale=-0.5,
            )

            # ctx idxs for the partials kv-writeback; on Pool so the prep
            # (also Pool) is ordered after it without a semaphore
            idx_t = cpool.tile([128, 1], mybir.dt.int32)
            nc.gpsimd.memset(idx_t[:], 0)

            def _w(binst):
                return binst

            # main stream
            s0 = 0
            for c, (csz, lane) in enumerate(plan):
                if c == 0:
                    t = t0
                else:
                    t = loads.tile([H, maxck, W], f32, tag="loads")
                    nc.sync.dma_start(
                        out=t[:, :csz, :],
                        in_=pred[s0 : s0 + csz].rearrange("s h w -> h s w"),
                    )

                # s1 column: sum over (s, w) of pred^2, per partition h
                if lane == "act":
                    sq = sqpool.tile([H, maxck, W], f32, tag="sq")
                    _w(nc.scalar.activation(
                        out=sq[:, :csz, :],
                        in_=t[:, :csz, :],
                        func=mybir.ActivationFunctionType.Square,
                        accum_out=outcols[:, 0, 0, c : c + 1],
                    ))
                else:
                    sq = sqpool.tile([H, maxck, W], f32, tag="sq")
                    nc.vector.tensor_mul(sq[:, :csz, :], t[:, :csz, :], t[:, :csz, :])
                    _w(nc.vector.reduce_sum(
                        outcols[:, 0, 0, c : c + 1],
                        sq[:, :csz, :],
                        axis=mybir.AxisListType.XY,
                    ))

                # s2 column: per-slice m' = pred_s^T @ gy_s, then dot gx
                ps = psumpool.tile([128, maxck], f32, tag="psum")
                for sj in range(csz):
                    s = s0 + sj
                    nc.tensor.matmul(
                        ps[:, sj : sj + 1],
                        t[:, sj, :],
                        gyx[:, s : s + 1],
                        start=True,
                        stop=True,
                    )
                prod = prodpool.tile([128, maxck], f32, tag="prod")
                nc.vector.tensor_mul(
                    prod[:, :csz], ps[:, :csz], gyx[:, SLICES + s0 : SLICES + s0 + csz]
                )
                _w(nc.vector.reduce_sum(
                    outcols[:, 0, 0, nck + c : nck + c + 1],
                    prod[:, :csz],
                    axis=mybir.AxisListType.X,
                ))
                s0 += csz

            # partials writeback: prep + trigger at the end (Tile's native
            # ordering computes sound writer waits). Post-finalize surgery
            # moves those waits from before the prep onto the trigger, so
            # the prep's ~1us SWDGE desc-gen runs early in wall-clock (Pool
            # is idle) instead of sitting on the tail critical path.
            dma_sem = nc.alloc_semaphore("pout_dma")
            prep = nc.gpsimd.kv_writeback(
                out_ap=partials[:, :, :, :],
                in_ap=outcols[:, :, :, :],
                ctx_idxs_ap=idx_t[:],
                prepare_only=True,
                sem=dma_sem,
            )
            prep_name = prep.ins.name
            trig = nc.gpsimd.trigger_dma(count=None)
            trig_name = trig.ins.name

    nc.finalize()

    fn = nc.m.functions[0]
    all_ins = [i for blk in fn.blocks for i in blk.instructions]

    # (1) Tile ticks the prep on the DMASW0 lane (the end-of-kernel drain
    # waits DMASW0 >= 16) but leaves our custom sem in on_update[0] — the
    # slot whose sem the SDMA bumps on completion. Point it at the DMASW0
    # sem so the drain accounting closes, matching the wiring a
    # non-prepared kv_writeback gets.
    dmasw = None
    for ins in all_ins:
        si = ins.sync_info
        if not si:
            continue
        for w in si.on_wait or []:
            if (w.ant_name or "").startswith("DMASW"):
                dmasw = w
                break
        if dmasw:
            break
    assert dmasw is not None, "no DMASW drain wait found"
    for ins in all_ins:
        if type(ins).__name__ == "InstKVWritebackAnt" and ins.gen_mode == 1:
            si = ins.sync_info
            upds = list(si.on_update)
            assert upds and upds[0].ant_name == "pout_dma", upds
            upds[0] = mybir.SyncUpdate(
                sync_type="semaphore",
                id=dmasw.id,
                ant_name=dmasw.ant_name,
                update_mode="sem-add-imm",
                update_value=16,
                update_reg=None,
            )
            si.on_update = upds

    # (2) relocate the writer-gating waits from before the prep onto the
    # trigger. Tile serializes Pool as [..., EventSem(DVE_tick >= M),
    # prep(wait Act_tick >= K), trigger(wait Pool_tick >= 1)]: the waits are
    # sound but force the prep's ~1us desc-gen to run after all compute.
    # Moving them to the trigger (same in-order queue, still before the DMA
    # fires) lets the prep run early; the trigger keeps the Pool-tick wait
    # so the doorbell still follows desc-gen completion.
    def _is_tick_wait(w):
        nm = w.ant_name or ""
        return (nm.startswith("Activation_") or nm.startswith("DVE_")) and \
            "sequencer" not in nm

    moved = []
    evsem = None
    for blk in fn.blocks:
        insl = blk.instructions
        trig_pos = next(
            (k for k, i in enumerate(insl) if i.name == trig_name), None
        )
        if trig_pos is None:
            continue
        ev_pos = None
        for k in range(trig_pos):
            ins = insl[k]
            if not str(ins.engine).endswith("Pool"):
                continue
            si = ins.sync_info
            if not si or not si.on_wait:
                continue
            hit = [w for w in si.on_wait if _is_tick_wait(w)]
            if not hit:
                continue
            moved.extend(hit)
            si.on_wait = [w for w in si.on_wait if not _is_tick_wait(w)]
            if type(ins).__name__ == "InstEventSemaphore":
                evsem = ins
                ev_pos = k
        assert len(moved) == 2 and evsem is not None, (moved, evsem)
        evsem.sync_info.on_wait = moved
        # move the (now 2-wait) EventSemaphore to just before the trigger so
        # the prep's desc-gen runs early while the gate still precedes the
        # DMA doorbell on the in-order Pool queue
        insl.pop(ev_pos)
        insl.insert(trig_pos - 1, evsem)
    return nc


def _joints_2d(proj_mats_batch, joints_3d_gt_batch):
    """Projected 2D joints [B, V, J, 2] (x, y) in float32 (reference math)."""
    joints = joints_3d_gt_batch.astype(np.float32)
    ones = np.ones(joints.shape[:-1] + (1,), dtype=np.float32)
    joints_h = np.concatenate([joints, ones], axis=-1)  # [B, J, 4]
    proj = np.einsum(
        "bvcd,bjd->bvjc", proj_mats_batch.astype(np.float32), joints_h
    ).astype(np.float32)  # [B, V, J, 3]
    return proj[..., :2] / proj[..., 2:3]


def kernel(heatmaps_pred, proj_mats_batch, joints_3d_gt_batch, joints_3d_valid_batch,
           _profile=None):
    heatmaps_pred = np.ascontiguousarray(np.asarray(heatmaps_pred, dtype=np.float32))
    joints_2d = _joints_2d(np.asarray(proj_mats_batch), np.asarray(joints_3d_gt_batch))

    # s3 = sum over slices of (sum_h gy^2) * (sum_w gx^2), exact in f64
    xs = np.arange(W, dtype=np.float64)
    ys = np.arange(H, dtype=np.float64)
    dx2 = (xs - joints_2d[..., 0, None].astype(np.float64)) ** 2  # [B,V,J,W]
    dy2 = (ys - joints_2d[..., 1, None].astype(np.float64)) ** 2  # [B,V,J,H]
    gx2 = np.exp(-dx2)  # gx^2 = exp(-dx2) for sigma=1
    gy2 = np.exp(-dy2)
    s3 = float((gy2.sum(-1) * gx2.sum(-1)).sum())

    if "nc" not in _CACHE:
        _CACHE["nc"] = _build_nc()
    nc = _CACHE["nc"]

    hramp = np.arange(128, dtype=np.float32)
    in_maps = []
    for c in range(N_CORES):
        bsl = slice(B_LOC * c, B_LOC * (c + 1))
        # slice order: (b_local, v, j) -> s
        y = joints_2d[bsl, :, :, 1].reshape(SLICES).astype(np.float32)
        x = joints_2d[bsl, :, :, 0].reshape(SLICES).astype(np.float32)
        consts = np.empty((2, 128 + 2 * SLICES), dtype=np.float32)
        consts[0, 0:128] = hramp
        consts[1, 0:128] = 1.0
        consts[0, 128:] = 1.0
        consts[1, 128 : 128 + SLICES] = -y
        consts[1, 128 + SLICES :] = -x
        in_maps.append(
            {
                "pred": heatmaps_pred[bsl].reshape(SLICES, H, W),
                "consts": consts,
            }
        )

    res = run_bass_kernel_spmd(nc, in_maps, core_ids=list(range(N_CORES)))
    if _profile is not None:
        _profile["result"] = res
        _profile["in_maps"] = in_maps

    s1 = 0.0
    s2 = 0.0
    for c in range(N_CORES):
        p = res.results[c]["partials"].astype(np.float64)  # [1, 128, 1, 2*NCK]
        s1 += p[0, :, 0, :NCK].sum()
        s2 += p[0, :, 0, NCK:].sum()

    total = s1 - 2.0 * s2 + s3
    return np.float32(total / (B * V * J * H * W))


# revision 32
# speedup vs baseline: 1.0609x; 1.0037x over previous
"""HeatmapMSELoss Trainium2 kernel.

Computes mean((heatmaps_pred - heatmaps_gt)^2) where heatmaps_gt is an
isotropic 2D gaussian (sigma=1, peak 1) rendered at the projection of each
3D joint into each view.

Key identity: the gaussian separates, gt[h,w] = gy[h] * gx[w], so

  sum_hw (pred - gt)^2 = sum_hw pred^2 - 2 * gy^T (pred @ gx) + (sum gy^2)(sum gx^2)

The 142MB gt tensor is never materialized. Per (b,v,j) slice the device
computes sum(pred^2) (scalar-engine square + accumulate, with a DVE lane for
part of the tail) and m' = pred^T @ gy (one matmul, PSUM column), then a
fused DVE multiply+reduce against gx. The 1D gaussians are generated ON
DEVICE from a tiny [2, 672] host constant (outer-product matmul for h - y,
then square + exp on DVE/ACT); the scalar combine runs on host in float64.

The per-chunk partials leave via a SWDGE kv-writeback whose descriptors are
prepared early on the idle Pool engine; an end-of-kernel trigger fires them,
skipping the HWDGE+DGE init latency on the tail critical path. Chunk sizes
taper at the end so the last chunks' compute (gated at DMA+900ns) doesn't
trail the final transfer.

Sharding: data-parallel over batch, 4 batches per core across 8 cores.
"""

import numpy as np

import concourse.bacc as bacc
import concourse.bass as bass
import concourse.tile as tile
from concourse import mybir
from concourse.bass_utils import run_bass_kernel_spmd

B, V, J, H, W = 32, 4, 17, 128, 128
SIGMA = 1.0
N_CORES = 8
B_LOC = B // N_CORES          # 4 batches per core
GROUPS = B_LOC * V            # 16 (b,v) groups per core
SLICES = GROUPS * J           # 272 slices per core

# stream plan: (chunk size in slices, engine lane for the sum-of-squares).
# Bulk 17-slice chunks on ACT (1-pass square+accum), tapered tail so the
# last chunks' compute latency (900ns DMA-sem gate + engine time) doesn't
# trail the final transfer; one DVE chunk near the end unloads ACT.
PLAN = [(17, "act")] * 13 + [(13, "act")] + [
    (10, "act"), (8, "act"), (8, "act"), (5, "act"), (3, "dve"), (4, "act")
]
assert sum(n for n, _ in PLAN) == SLICES
NCK = len(PLAN)

_CACHE = {}


def _build_nc(plan=None, load_bufs=6, tail_cross=5):
    nc = bacc.Bacc()
    f32 = mybir.dt.float32
    plan = list(PLAN) if plan is None else list(plan)
    chunks = [c for c, _ in plan]
    nck = len(chunks)
    maxck = max(chunks)
    # chunks whose cross-term mul+reduce is batched into one DVE op pair
    # (their matmuls share one PSUM tile); one extra s2 output column.
    # Output columns: [0, nck) s1 per chunk; [nck, nck+tc0) s2 per non-tail
    # chunk; [nck+tc0] the batched tail s2.
    tc0 = nck - tail_cross
    tail_slices = sum(chunks[tc0:])
    NCN = nck + tc0 + 1

    pred = nc.declare_dram_parameter("pred", [SLICES, H, W], f32, isOutput=False)
    consts = nc.declare_dram_parameter(
        "consts", [2, 128 + 2 * SLICES], f32, isOutput=False
    )
    partials = nc.declare_dram_parameter(
        "partials", [1, 128, 1, NCN], f32, isOutput=True
    )

    with tile.TileContext(nc) as tc:
        with (
            tc.tile_pool(name="cpool", bufs=1) as cpool,
            tc.tile_pool(name="loads", bufs=load_bufs) as loads,
            tc.tile_pool(name="sq", bufs=2) as sqpool,
            tc.tile_pool(name="prod", bufs=2) as prodpool,
            tc.tile_pool(name="psum", bufs=4, space="PSUM") as psumpool,
            tc.tile_pool(name="gpsum", bufs=1, space="PSUM") as gpsumpool,
            tc.tile_pool(name="tailpsum", bufs=1, space="PSUM") as tailpsumpool,
            tc.tile_pool(name="outs", bufs=1) as outs,
        ):
            # ACT table warm-up (Square+Exp share a table set); the load
            # overlaps the first pred DMA
            warm = cpool.tile([128, 1], f32)
            nc.vector.memset(warm[:], 0.0)
            wsq = cpool.tile([128, 1], f32)
            nc.scalar.activation(
                out=wsq[:], in_=warm[:], func=mybir.ActivationFunctionType.Square
            )
            nc.scalar.activation(
                out=wsq[:], in_=warm[:], func=mybir.ActivationFunctionType.Exp
            )

            outcols = outs.tile([128, 1, 1, NCN], f32)

            # chunk 0 pred DMA issued FIRST so the big stream starts asap
            c0 = chunks[0]
            t0 = loads.tile([H, maxck, W], f32, tag="loads")
            nc.sync.dma_start(
                out=t0[:, :c0, :],
                in_=pred[0:c0].rearrange("s h w -> h s w"),
            )

            # consts DMA + on-device gaussian generation:
            # gy[h,s] = exp(-0.5 (h - y_s)^2), gx likewise, via an
            # outer-product matmul (dy = h - y) then square (DVE) + exp (ACT)
            ct = cpool.tile([2, 128 + 2 * SLICES], f32)
            nc.sync.dma_start(out=ct[:], in_=consts[:, :])
            psy = gpsumpool.tile([128, SLICES], f32)
            nc.tensor.matmul(
                psy[:], ct[:, 0:128], ct[:, 128 : 128 + SLICES], start=True, stop=True
            )
            psx = gpsumpool.tile([128, SLICES], f32)
            nc.tensor.matmul(
                psx[:],
                ct[:, 0:128],
                ct[:, 128 + SLICES : 128 + 2 * SLICES],
                start=True,
                stop=True,
            )
            gsq = cpool.tile([128, 2 * SLICES], f32)
            nc.scalar.activation(
                out=gsq[:, 0:SLICES], in_=psy[:],
                func=mybir.ActivationFunctionType.Square,
            )
            nc.scalar.activation(
                out=gsq[:, SLICES:], in_=psx[:],
                func=mybir.ActivationFunctionType.Square,
            )
            gyx = cpool.tile([128, 2 * SLICES], f32)
            nc.scalar.activation(
                out=gyx[:],
                in_=gsq[:],
                func=mybir.ActivationFunctionType.Exp,
                scale=-0.5,
            )

            # ctx idxs for the partials kv-writeback; on Pool so the prep
            # (also Pool) is ordered after it without a semaphore
            idx_t = cpool.tile([128, 1], mybir.dt.int32)
            nc.gpsimd.memset(idx_t[:], 0)

            def _w(binst):
                return binst

            # main stream
            s0 = 0
            for c, (csz, lane) in enumerate(plan):
                if c == 0:
                    t = t0
                else:
                    t = loads.tile([H, maxck, W], f32, tag="loads")
                    nc.sync.dma_start(
                        out=t[:, :csz, :],
                        in_=pred[s0 : s0 + csz].rearrange("s h w -> h s w"),
                    )

                # s1 column: sum over (s, w) of pred^2, per partition h
                if lane in ("act", "x"):
                    sq = sqpool.tile([H, maxck, W], f32, tag="sq")
                    _w(nc.scalar.activation(
                        out=sq[:, :csz, :],
                        in_=t[:, :csz, :],
                        func=mybir.ActivationFunctionType.Square,
                        accum_out=outcols[:, 0, 0, c : c + 1],
                    ))
                else:
                    # square on Pool ('pd') or DVE ('dve'); reduce on DVE
                    sqeng = nc.gpsimd if lane == "pd" else nc.vector
                    sq = sqpool.tile([H, maxck, W], f32, tag="sq")
                    sqeng.tensor_mul(sq[:, :csz, :], t[:, :csz, :], t[:, :csz, :])
                    _w(nc.vector.reduce_sum(
                        outcols[:, 0, 0, c : c + 1],
                        sq[:, :csz, :],
                        axis=mybir.AxisListType.XY,
                    ))

                # s2: per-slice m' = pred_s^T @ gy_s (PSUM column)
                if c == tc0:
                    tailps = tailpsumpool.tile(
                        [128, tail_slices], f32, tag="tailps"
                    )
                    tail_s0 = s0
                if c >= tc0:
                    ps = tailps
                    poff = s0 - tail_s0
                else:
                    ps = psumpool.tile([128, maxck], f32, tag="psum")
                    poff = 0
                for sj in range(csz):
                    s = s0 + sj
                    nc.tensor.matmul(
                        ps[:, poff + sj : poff + sj + 1],
                        t[:, sj, :],
                        gyx[:, s : s + 1],
                        start=True,
                        stop=True,
                    )
                if c < tc0:
                    prod = prodpool.tile([128, maxck], f32, tag="prod")
                    nc.vector.tensor_mul(
                        prod[:, :csz],
                        ps[:, :csz],
                        gyx[:, SLICES + s0 : SLICES + s0 + csz],
                    )
                    _w(nc.vector.reduce_sum(
                        outcols[:, 0, 0, nck + c : nck + c + 1],
                        prod[:, :csz],
                        axis=mybir.AxisListType.X,
                    ))
                s0 += csz

            # batched cross for the tail chunks: one DVE mul + one reduce
            # over all their PSUM columns, into the extra s2 column
            tprod = prodpool.tile([128, tail_slices], f32, tag="tprod")
            nc.vector.tensor_mul(
                tprod[:],
                tailps[:],
                gyx[:, SLICES + tail_s0 : SLICES + tail_s0 + tail_slices],
            )
            _w(nc.vector.reduce_sum(
                outcols[:, 0, 0, nck + tc0 : nck + tc0 + 1],
                tprod[:],
                axis=mybir.AxisListType.X,
            ))

            # partials writeback: prep + trigger at the end (Tile's native
            # ordering computes sound writer waits). Post-finalize surgery
            # moves those waits from before the prep onto the trigger, so
            # the prep's ~1us SWDGE desc-gen runs early in wall-clock (Pool
            # is idle) instead of sitting on the tail critical path.
            dma_sem = nc.alloc_semaphore("pout_dma")
            prep = nc.gpsimd.kv_writeback(
                out_ap=partials[:, :, :, :],
                in_ap=outcols[:, :, :, :],
                ctx_idxs_ap=idx_t[:],
                prepare_only=True,
                sem=dma_sem,
            )
            prep_name = prep.ins.name
            trig = nc.gpsimd.trigger_dma(count=None)
            trig_name = trig.ins.name

    nc.finalize()

    fn = nc.m.functions[0]
    all_ins = [i for blk in fn.blocks for i in blk.instructions]

    # (1) Tile ticks the prep on the DMASW0 lane (the end-of-kernel drain
    # waits DMASW0 >= 16) but leaves our custom sem in on_update[0] — the
    # slot whose sem the SDMA bumps on completion. Point it at the DMASW0
    # sem so the drain accounting closes, matching the wiring a
    # non-prepared kv_writeback gets.
    dmasw = None
    for ins in all_ins:
        si = ins.sync_info
        if not si:
            continue
        for w in si.on_wait or []:
            if (w.ant_name or "").startswith("DMASW"):
                dmasw = w
                break
        if dmasw:
            break
    assert dmasw is not None, "no DMASW drain wait found"
    for ins in all_ins:
        if type(ins).__name__ == "InstKVWritebackAnt" and ins.gen_mode == 1:
            si = ins.sync_info
            upds = list(si.on_update)
            assert upds and upds[0].ant_name == "pout_dma", upds
            upds[0] = mybir.SyncUpdate(
                sync_type="semaphore",
                id=dmasw.id,
                ant_name=dmasw.ant_name,
                update_mode="sem-add-imm",
                update_value=16,
                update_reg=None,
            )
            si.on_update = upds

    # (2) relocate the writer-gating waits from before the prep onto the
    # trigger. Tile serializes Pool as [..., EventSem(DVE_tick >= M),
    # prep(wait Act_tick >= K), trigger(wait Pool_tick >= 1)]: the waits are
    # sound but force the prep's ~1us desc-gen to run after all compute.
    # Moving them to the trigger (same in-order queue, still before the DMA
    # fires) lets the prep run early; the trigger keeps the Pool-tick wait
    # so the doorbell still follows desc-gen completion.
    def _is_tick_wait(w):
        nm = w.ant_name or ""
        return (nm.startswith("Activation_") or nm.startswith("DVE_")) and \
            "sequencer" not in nm

    moved = []
    evsem = None
    for blk in fn.blocks:
        insl = blk.instructions
        trig_pos = next(
            (k for k, i in enumerate(insl) if i.name == trig_name), None
        )
        if trig_pos is None:
            continue
        prep_pos = next(k for k, i in enumerate(insl) if i.name == prep_name)
        # strip gating waits from the prep and from the contiguous
        # bookkeeping block (EventSemaphore / ring setup) just before it —
        # NOT from earlier pool compute, whose waits are load-bearing
        strip = [prep_pos]
        k = prep_pos - 1
        while k >= 0 and type(insl[k]).__name__ in (
            "InstEventSemaphore",
            "InstPseudoReloadLibraryIndex",
            "InstIncSwdgeSem",
        ):
            strip.append(k)
            k -= 1
        ev_pos = None
        for k in strip:
            ins = insl[k]
            si = ins.sync_info
            if not si or not si.on_wait:
                continue
            hit = [w for w in si.on_wait if _is_tick_wait(w)]
            if not hit:
                continue
            moved.extend(hit)
            si.on_wait = [w for w in si.on_wait if not _is_tick_wait(w)]
            if type(ins).__name__ == "InstEventSemaphore":
                evsem = ins
                ev_pos = k
        if evsem is None:
            # no EventSemaphore in the prep block carried gating waits;
            # borrow the last bookkeeping EventSemaphore before the prep
            ev_pos = next(
                k for k in strip
                if type(insl[k]).__name__ == "InstEventSemaphore"
            )
            evsem = insl[ev_pos]
        # dedupe by semaphore, keeping the strongest condition
        bysem = {}
        for w in moved:
            if w.id not in bysem or (w.wait_value or 0) > (
                bysem[w.id].wait_value or 0
            ):
                bysem[w.id] = w
        moved = list(bysem.values())
        assert 1 <= len(moved) <= 2, moved
        evsem.sync_info.on_wait = moved
        # move the gate EventSemaphore to just before the trigger so the
        # prep's desc-gen runs early while the gate still precedes the DMA
        # doorbell on the in-order Pool queue
        insl.pop(ev_pos)
        trig_pos = next(k for k, i in enumerate(insl) if i.name == trig_name)
        insl.insert(trig_pos, evsem)
    return nc


def _joints_2d(proj_mats_batch, joints_3d_gt_batch):
    """Projected 2D joints [B, V, J, 2] (x, y) in float32 (reference math)."""
    joints = joints_3d_gt_batch.astype(np.float32)
    ones = np.ones(joints.shape[:-1] + (1,), dtype=np.float32)
    joints_h = np.concatenate([joints, ones], axis=-1)  # [B, J, 4]
    proj = np.einsum(
        "bvcd,bjd->bvjc", proj_mats_batch.astype(np.float32), joints_h
    ).astype(np.float32)  # [B, V, J, 3]
    return proj[..., :2] / proj[..., 2:3]


def kernel(heatmaps_pred, proj_mats_batch, joints_3d_gt_batch, joints_3d_valid_batch,
           _profile=None):
    heatmaps_pred = np.ascontiguousarray(np.asarray(heatmaps_pred, dtype=np.float32))
    joints_2d = _joints_2d(np.asarray(proj_mats_batch), np.asarray(joints_3d_gt_batch))

    # s3 = sum over slices of (sum_h gy^2) * (sum_w gx^2), exact in f64
    xs = np.arange(W, dtype=np.float64)
    ys = np.arange(H, dtype=np.float64)
    dx2 = (xs - joints_2d[..., 0, None].astype(np.float64)) ** 2  # [B,V,J,W]
    dy2 = (ys - joints_2d[..., 1, None].astype(np.float64)) ** 2  # [B,V,J,H]
    gx2 = np.exp(-dx2)  # gx^2 = exp(-dx2) for sigma=1
    gy2 = np.exp(-dy2)
    s3 = float((gy2.sum(-1) * gx2.sum(-1)).sum())

    if "nc" not in _CACHE:
        _CACHE["nc"] = _build_nc()
    nc = _CACHE["nc"]

    hramp = np.arange(128, dtype=np.float32)
    in_maps = []
    for c in range(N_CORES):
        bsl = slice(B_LOC * c, B_LOC * (c + 1))
        # slice order: (b_local, v, j) -> s
        y = joints_2d[bsl, :, :, 1].reshape(SLICES).astype(np.float32)
        x = joints_2d[bsl, :, :, 0].reshape(SLICES).astype(np.float32)
        consts = np.empty((2, 128 + 2 * SLICES), dtype=np.float32)
        consts[0, 0:128] = hramp
        consts[1, 0:128] = 1.0
        consts[0, 128:] = 1.0
        consts[1, 128 : 128 + SLICES] = -y
        consts[1, 128 + SLICES :] = -x
        in_maps.append(
            {
                "pred": heatmaps_pred[bsl].reshape(SLICES, H, W),
                "consts": consts,
            }
        )

    res = run_bass_kernel_spmd(nc, in_maps, core_ids=list(range(N_CORES)))
    if _profile is not None:
        _profile["result"] = res
        _profile["in_maps"] = in_maps

    s1 = 0.0
    s2 = 0.0
    for c in range(N_CORES):
        p = res.results[c]["partials"].astype(np.float64)  # [1, 128, 1, 2*NCK]
        s1 += p[0, :, 0, :NCK].sum()
        s2 += p[0, :, 0, NCK:].sum()

    total = s1 - 2.0 * s2 + s3
    return np.float32(total / (B * V * J * H * W))


# revision 34
# speedup vs baseline: 1.0763x; 1.0146x over previous
"""HeatmapMSELoss Trainium2 kernel.

Computes mean((heatmaps_pred - heatmaps_gt)^2) where heatmaps_gt is an
isotropic 2D gaussian (sigma=1, peak 1) rendered at the projection of each
3D joint into each view.

Key identity: the gaussian separates, gt[h,w] = gy[h] * gx[w], so

  sum_hw (pred - gt)^2 = sum_hw pred^2 - 2 * gy^T (pred @ gx) + (sum gy^2)(sum gx^2)

The 142MB gt tensor is never materialized. Per (b,v,j) slice the device
computes sum(pred^2) (scalar-engine square + accumulate, with a DVE lane for
part of the tail) and m' = pred^T @ gy (one matmul, PSUM column), then a
fused DVE multiply+reduce against gx. The 1D gaussians are generated ON
DEVICE from a tiny [2, 672] host constant (outer-product matmul for h - y,
then square + exp on DVE/ACT); the scalar combine runs on host in float64.

The per-chunk partials leave via a SWDGE kv-writeback whose descriptors are
prepared early on the idle Pool engine; an end-of-kernel trigger fires them,
skipping the HWDGE+DGE init latency on the tail critical path. Chunk sizes
taper at the end so the last chunks' compute (gated at DMA+900ns) doesn't
trail the final transfer.

Sharding: data-parallel over batch, 4 batches per core across 8 cores.
"""

import numpy as np

import concourse.bacc as bacc
import concourse.bass as bass
import concourse.tile as tile
from concourse import mybir
from concourse.bass_utils import run_bass_kernel_spmd

B, V, J, H, W = 32, 4, 17, 128, 128
SIGMA = 1.0
N_CORES = 8
B_LOC = B // N_CORES          # 4 batches per core
GROUPS = B_LOC * V            # 16 (b,v) groups per core
SLICES = GROUPS * J           # 272 slices per core

# stream plan: (chunk size in slices, engine lane for the sum-of-squares).
# Bulk 17-slice chunks on ACT (1-pass square+accum), tapered tail so the
# last chunks' compute latency (900ns DMA-sem gate + engine time) doesn't
# trail the final transfer; one DVE chunk near the end unloads ACT.
PLAN = [(17, "act")] * 13 + [(13, "act")] + [
    (10, "act"), (8, "act"), (8, "act"), (5, "act"), (3, "dve"), (4, "act")
]
assert sum(n for n, _ in PLAN) == SLICES
NCK = len(PLAN)

_CACHE = {}


def _build_nc(plan=None, load_bufs=6, tail_cross=5):
    nc = bacc.Bacc()
    f32 = mybir.dt.float32
    plan = list(PLAN) if plan is None else list(plan)
    chunks = [c for c, _ in plan]
    nck = len(chunks)
    maxck = max(chunks)
    # chunks whose cross-term mul+reduce is batched into one DVE op pair
    # (their matmuls share one PSUM tile); one extra s2 output column.
    # Output columns: [0, nck) s1 per chunk; [nck, nck+tc0) s2 per non-tail
    # chunk; [nck+tc0] the batched tail s2.
    tc0 = nck - tail_cross
    tail_slices = sum(chunks[tc0:])
    NCN = nck + tc0 + 1

    pred = nc.declare_dram_parameter("pred", [SLICES, H, W], f32, isOutput=False)
    consts = nc.declare_dram_parameter(
        "consts", [2, 128 + 2 * SLICES], f32, isOutput=False
    )
    partials = nc.declare_dram_parameter(
        "partials", [1, 128, 1, NCN], f32, isOutput=True
    )

    with tile.TileContext(nc) as tc:
        with (
            tc.tile_pool(name="cpool", bufs=1) as cpool,
            tc.tile_pool(name="loads", bufs=load_bufs) as loads,
            tc.tile_pool(name="sq", bufs=2) as sqpool,
            tc.tile_pool(name="prod", bufs=2) as prodpool,
            tc.tile_pool(name="psum", bufs=4, space="PSUM") as psumpool,
            tc.tile_pool(name="gpsum", bufs=1, space="PSUM") as gpsumpool,
            tc.tile_pool(name="tailpsum", bufs=1, space="PSUM") as tailpsumpool,
            tc.tile_pool(name="outs", bufs=1) as outs,
        ):
            # ACT table warm-up (Square+Exp share a table set); the load
            # overlaps the first pred DMA
            warm = cpool.tile([128, 1], f32)
            nc.vector.memset(warm[:], 0.0)
            wsq = cpool.tile([128, 1], f32)
            nc.scalar.activation(
                out=wsq[:], in_=warm[:], func=mybir.ActivationFunctionType.Square
            )
            nc.scalar.activation(
                out=wsq[:], in_=warm[:], func=mybir.ActivationFunctionType.Exp
            )

            outcols = outs.tile([128, 1, 1, NCN], f32)

            # chunk 0 pred DMA issued FIRST so the big stream starts asap
            c0 = chunks[0]
            t0 = loads.tile([H, maxck, W], f32, tag="loads")
            dma0 = nc.sync.dma_start(
                out=t0[:, :c0, :],
                in_=pred[0:c0].rearrange("s h w -> h s w"),
            )
            dma0_name = dma0.ins.name

            # consts DMA + on-device gaussian generation:
            # gy[h,s] = exp(-0.5 (h - y_s)^2), gx likewise, via an
            # outer-product matmul (dy = h - y) then square (DVE) + exp (ACT)
            ct = cpool.tile([2, 128 + 2 * SLICES], f32)
            nc.sync.dma_start(out=ct[:], in_=consts[:, :])
            psy = gpsumpool.tile([128, SLICES], f32)
            nc.tensor.matmul(
                psy[:], ct[:, 0:128], ct[:, 128 : 128 + SLICES], start=True, stop=True
            )
            psx = gpsumpool.tile([128, SLICES], f32)
            nc.tensor.matmul(
                psx[:],
                ct[:, 0:128],
                ct[:, 128 + SLICES : 128 + 2 * SLICES],
                start=True,
                stop=True,
            )
            gsq = cpool.tile([128, 2 * SLICES], f32)
            nc.scalar.activation(
                out=gsq[:, 0:SLICES], in_=psy[:],
                func=mybir.ActivationFunctionType.Square,
            )
            nc.scalar.activation(
                out=gsq[:, SLICES:], in_=psx[:],
                func=mybir.ActivationFunctionType.Square,
            )
            gyx = cpool.tile([128, 2 * SLICES], f32)
            nc.scalar.activation(
                out=gyx[:],
                in_=gsq[:],
                func=mybir.ActivationFunctionType.Exp,
                scale=-0.5,
            )

            # ctx idxs for the partials kv-writeback; on Pool so the prep
            # (also Pool) is ordered after it without a semaphore
            idx_t = cpool.tile([128, 1], mybir.dt.int32)
            nc.gpsimd.memset(idx_t[:], 0)

            def _w(binst):
                return binst

            # main stream
            s0 = 0
            for c, (csz, lane) in enumerate(plan):
                if c == 0:
                    t = t0
                else:
                    t = loads.tile([H, maxck, W], f32, tag="loads")
                    nc.sync.dma_start(
                        out=t[:, :csz, :],
                        in_=pred[s0 : s0 + csz].rearrange("s h w -> h s w"),
                    )

                # s1 column: sum over (s, w) of pred^2, per partition h
                if lane in ("act", "x"):
                    sq = sqpool.tile([H, maxck, W], f32, tag="sq")
                    _w(nc.scalar.activation(
                        out=sq[:, :csz, :],
                        in_=t[:, :csz, :],
                        func=mybir.ActivationFunctionType.Square,
                        accum_out=outcols[:, 0, 0, c : c + 1],
                    ))
                else:
                    # square on Pool ('pd') or DVE ('dve'); reduce on DVE
                    sqeng = nc.gpsimd if lane == "pd" else nc.vector
                    sq = sqpool.tile([H, maxck, W], f32, tag="sq")
                    sqeng.tensor_mul(sq[:, :csz, :], t[:, :csz, :], t[:, :csz, :])
                    _w(nc.vector.reduce_sum(
                        outcols[:, 0, 0, c : c + 1],
                        sq[:, :csz, :],
                        axis=mybir.AxisListType.XY,
                    ))

                # s2: per-slice m' = pred_s^T @ gy_s (PSUM column)
                if c == tc0:
                    tailps = tailpsumpool.tile(
                        [128, tail_slices], f32, tag="tailps"
                    )
                    tail_s0 = s0
                if c >= tc0:
                    ps = tailps
                    poff = s0 - tail_s0
                else:
                    ps = psumpool.tile([128, maxck], f32, tag="psum")
                    poff = 0
                for sj in range(csz):
                    s = s0 + sj
                    nc.tensor.matmul(
                        ps[:, poff + sj : poff + sj + 1],
                        t[:, sj, :],
                        gyx[:, s : s + 1],
                        start=True,
                        stop=True,
                    )
                if c < tc0:
                    prod = prodpool.tile([128, maxck], f32, tag="prod")
                    nc.vector.tensor_mul(
                        prod[:, :csz],
                        ps[:, :csz],
                        gyx[:, SLICES + s0 : SLICES + s0 + csz],
                    )
                    _w(nc.vector.reduce_sum(
                        outcols[:, 0, 0, nck + c : nck + c + 1],
                        prod[:, :csz],
                        axis=mybir.AxisListType.X,
                    ))
                s0 += csz

            # batched cross for the tail chunks: one DVE mul + one reduce
            # over all their PSUM columns, into the extra s2 column
            tprod = prodpool.tile([128, tail_slices], f32, tag="tprod")
            nc.vector.tensor_mul(
                tprod[:],
                tailps[:],
                gyx[:, SLICES + tail_s0 : SLICES + tail_s0 + tail_slices],
            )
            _w(nc.vector.reduce_sum(
                outcols[:, 0, 0, nck + tc0 : nck + tc0 + 1],
                tprod[:],
                axis=mybir.AxisListType.X,
            ))

            # partials writeback: prep + trigger at the end (Tile's native
            # ordering computes sound writer waits). Post-finalize surgery
            # moves those waits from before the prep onto the trigger, so
            # the prep's ~1us SWDGE desc-gen runs early in wall-clock (Pool
            # is idle) instead of sitting on the tail critical path.
            dma_sem = nc.alloc_semaphore("pout_dma")
            prep = nc.gpsimd.kv_writeback(
                out_ap=partials[:, :, :, :],
                in_ap=outcols[:, :, :, :],
                ctx_idxs_ap=idx_t[:],
                prepare_only=True,
                sem=dma_sem,
            )
            prep_name = prep.ins.name
            trig = nc.gpsimd.trigger_dma(count=None)
            trig_name = trig.ins.name

    nc.finalize()

    fn = nc.m.functions[0]
    all_ins = [i for blk in fn.blocks for i in blk.instructions]

    # (1) Tile ticks the prep on the DMASW0 lane (the end-of-kernel drain
    # waits DMASW0 >= 16) but leaves our custom sem in on_update[0] — the
    # slot whose sem the SDMA bumps on completion. Point it at the DMASW0
    # sem so the drain accounting closes, matching the wiring a
    # non-prepared kv_writeback gets.
    dmasw = None
    for ins in all_ins:
        si = ins.sync_info
        if not si:
            continue
        for w in si.on_wait or []:
            if (w.ant_name or "").startswith("DMASW"):
                dmasw = w
                break
        if dmasw:
            break
    assert dmasw is not None, "no DMASW drain wait found"
    for ins in all_ins:
        if type(ins).__name__ == "InstKVWritebackAnt" and ins.gen_mode == 1:
            si = ins.sync_info
            upds = list(si.on_update)
            assert upds and upds[0].ant_name == "pout_dma", upds
            upds[0] = mybir.SyncUpdate(
                sync_type="semaphore",
                id=dmasw.id,
                ant_name=dmasw.ant_name,
                update_mode="sem-add-imm",
                update_value=16,
                update_reg=None,
            )
            si.on_update = upds

    # (2) relocate the writer-gating waits from before the prep onto the
    # trigger. Tile serializes Pool as [..., EventSem(DVE_tick >= M),
    # prep(wait Act_tick >= K), trigger(wait Pool_tick >= 1)]: the waits are
    # sound but force the prep's ~1us desc-gen to run after all compute.
    # Moving them to the trigger (same in-order queue, still before the DMA
    # fires) lets the prep run early; the trigger keeps the Pool-tick wait
    # so the doorbell still follows desc-gen completion.
    def _is_tick_wait(w):
        nm = w.ant_name or ""
        return (nm.startswith("Activation_") or nm.startswith("DVE_")) and \
            "sequencer" not in nm

    moved = []
    evsem = None
    for blk in fn.blocks:
        insl = blk.instructions
        trig_pos = next(
            (k for k, i in enumerate(insl) if i.name == trig_name), None
        )
        if trig_pos is None:
            continue
        prep_pos = next(k for k, i in enumerate(insl) if i.name == prep_name)
        # strip gating waits from the prep and from the contiguous
        # bookkeeping block (EventSemaphore / ring setup) just before it —
        # NOT from earlier pool compute, whose waits are load-bearing
        strip = [prep_pos]
        k = prep_pos - 1
        while k >= 0 and type(insl[k]).__name__ in (
            "InstEventSemaphore",
            "InstPseudoReloadLibraryIndex",
            "InstIncSwdgeSem",
        ):
            strip.append(k)
            k -= 1
        ev_pos = None
        for k in strip:
            ins = insl[k]
            si = ins.sync_info
            if not si or not si.on_wait:
                continue
            hit = [w for w in si.on_wait if _is_tick_wait(w)]
            if not hit:
                continue
            moved.extend(hit)
            si.on_wait = [w for w in si.on_wait if not _is_tick_wait(w)]
            if type(ins).__name__ == "InstEventSemaphore":
                evsem = ins
                ev_pos = k
        if evsem is None:
            # no EventSemaphore in the prep block carried gating waits;
            # borrow the last bookkeeping EventSemaphore before the prep
            ev_pos = next(
                k for k in strip
                if type(insl[k]).__name__ == "InstEventSemaphore"
            )
            evsem = insl[ev_pos]
        # dedupe by semaphore, keeping the strongest condition
        bysem = {}
        for w in moved:
            if w.id not in bysem or (w.wait_value or 0) > (
                bysem[w.id].wait_value or 0
            ):
                bysem[w.id] = w
        moved = list(bysem.values())
        assert 1 <= len(moved) <= 2, moved
        evsem.sync_info.on_wait = moved
        # move the gate EventSemaphore to just before the trigger so the
        # prep's desc-gen runs early while the gate still precedes the DMA
        # doorbell on the in-order Pool queue
        insl.pop(ev_pos)
        trig_pos = next(k for k, i in enumerate(insl) if i.name == trig_name)
        insl.insert(trig_pos, evsem)

    # (3) hoist the first pred DMA into the preamble block, before SP's
    # entry-barrier EventSemaphore: its SEQ decode + HWDGE descriptor-gen
    # (~650ns) then overlap the all-engine entry barrier, so the first
    # transfer starts ~640ns earlier. The DMA has no waits and its target
    # tile is untouched by the preamble.
    blk0 = fn.blocks[0]
    dma0_ins = None
    for blk in fn.blocks:
        insl = blk.instructions
        for k, ins in enumerate(insl):
            if ins.name == dma0_name:
                dma0_ins = insl.pop(k)
                break
        if dma0_ins:
            break
    assert dma0_ins is not None
    bar_sp = next(
        k for k, i in enumerate(blk0.instructions)
        if type(i).__name__ == "InstEventSemaphore"
        and str(i.engine).endswith("SP")
    )
    blk0.instructions.insert(bar_sp, dma0_ins)

    # (4) the six exit drain EventSemaphores on SP serialize at ~50ns each;
    # put the DMASW0 (partials writeback) wait on the LAST of them so the
    # others complete during the stream instead of after the kv sem fires.
    exit_blk = fn.blocks[-1]
    drains = [
        i for i in exit_blk.instructions
        if type(i).__name__ == "InstEventSemaphore"
        and str(i.engine).endswith("SP")
        and i.sync_info and len(i.sync_info.on_wait or []) == 2
    ]
    if len(drains) >= 2:
        src = next(
            (d for d in drains
             if any((w.ant_name or "").startswith("DMASW")
                    for w in d.sync_info.on_wait)),
            None,
        )
        dst = drains[-1]
        if src is not None and src is not dst:
            sw = src.sync_info.on_wait
            dw = dst.sync_info.on_wait
            si = next(k for k, w in enumerate(sw)
                      if (w.ant_name or "").startswith("DMASW"))
            sw[si], dw[-1] = dw[-1], sw[si]
            src.sync_info.on_wait = sw
            dst.sync_info.on_wait = dw
    return nc


def _joints_2d(proj_mats_batch, joints_3d_gt_batch):
    """Projected 2D joints [B, V, J, 2] (x, y) in float32 (reference math)."""
    joints = joints_3d_gt_batch.astype(np.float32)
    ones = np.ones(joints.shape[:-1] + (1,), dtype=np.float32)
    joints_h = np.concatenate([joints, ones], axis=-1)  # [B, J, 4]
    proj = np.einsum(
        "bvcd,bjd->bvjc", proj_mats_batch.astype(np.float32), joints_h
    ).astype(np.float32)  # [B, V, J, 3]
    return proj[..., :2] / proj[..., 2:3]


def kernel(heatmaps_pred, proj_mats_batch, joints_3d_gt_batch, joints_3d_valid_batch,
           _profile=None):
    heatmaps_pred = np.ascontiguousarray(np.asarray(heatmaps_pred, dtype=np.float32))
    joints_2d = _joints_2d(np.asarray(proj_mats_batch), np.asarray(joints_3d_gt_batch))

    # s3 = sum over slices of (sum_h gy^2) * (sum_w gx^2), exact in f64
    xs = np.arange(W, dtype=np.float64)
    ys = np.arange(H, dtype=np.float64)
    dx2 = (xs - joints_2d[..., 0, None].astype(np.float64)) ** 2  # [B,V,J,W]
    dy2 = (ys - joints_2d[..., 1, None].astype(np.float64)) ** 2  # [B,V,J,H]
    gx2 = np.exp(-dx2)  # gx^2 = exp(-dx2) for sigma=1
    gy2 = np.exp(-dy2)
    s3 = float((gy2.sum(-1) * gx2.sum(-1)).sum())

    if "nc" not in _CACHE:
        _CACHE["nc"] = _build_nc()
    nc = _CACHE["nc"]

    hramp = np.arange(128, dtype=np.float32)
    in_maps = []
    for c in range(N_CORES):
        bsl = slice(B_LOC * c, B_LOC * (c + 1))
        # slice order: (b_local, v, j) -> s
        y = joints_2d[bsl, :, :, 1].reshape(SLICES).astype(np.float32)
        x = joints_2d[bsl, :, :, 0].reshape(SLICES).astype(np.float32)
        consts = np.empty((2, 128 + 2 * SLICES), dtype=np.float32)
        consts[0, 0:128] = hramp
        consts[1, 0:128] = 1.0
        consts[0, 128:] = 1.0
        consts[1, 128 : 128 + SLICES] = -y
        consts[1, 128 + SLICES :] = -x
        in_maps.append(
            {
                "pred": heatmaps_pred[bsl].reshape(SLICES, H, W),
                "consts": consts,
            }
        )

    res = run_bass_kernel_spmd(nc, in_maps, core_ids=list(range(N_CORES)))
    if _profile is not None:
        _profile["result"] = res
        _profile["in_maps"] = in_maps

    s1 = 0.0
    s2 = 0.0
    for c in range(N_CORES):
        p = res.results[c]["partials"].astype(np.float64)  # [1, 128, 1, 2*NCK]
        s1 += p[0, :, 0, :NCK].sum()
        s2 += p[0, :, 0, NCK:].sum()

    total = s1 - 2.0 * s2 + s3
    return np.float32(total / (B * V * J * H * W))


# revision 37
# speedup vs baseline: 1.0768x; 1.0005x over previous
"""HeatmapMSELoss Trainium2 kernel.

Computes mean((heatmaps_pred - heatmaps_gt)^2) where heatmaps_gt is an
isotropic 2D gaussian (sigma=1, peak 1) rendered at the projection of each
3D joint into each view.

Key identity: the gaussian separates, gt[h,w] = gy[h] * gx[w], so

  sum_hw (pred - gt)^2 = sum_hw pred^2 - 2 * gy^T (pred @ gx) + (sum gy^2)(sum gx^2)

The 142MB gt tensor is never materialized. Per (b,v,j) slice the device
computes sum(pred^2) (scalar-engine square + accumulate, with a DVE lane for
part of the tail) and m' = pred^T @ gy (one matmul, PSUM column), then a
fused DVE multiply+reduce against gx. The 1D gaussians are generated ON
DEVICE from a tiny [2, 672] host constant (outer-product matmul for h - y,
then square + exp on DVE/ACT); the scalar combine runs on host in float64.

The per-chunk partials leave via a SWDGE kv-writeback whose descriptors are
prepared early on the idle Pool engine; an end-of-kernel trigger fires them,
skipping the HWDGE+DGE init latency on the tail critical path. Chunk sizes
taper at the end so the last chunks' compute (gated at DMA+900ns) doesn't
trail the final transfer.

Sharding: data-parallel over batch, 4 batches per core across 8 cores.
"""

import numpy as np

import concourse.bacc as bacc
import concourse.bass as bass
import concourse.tile as tile
from concourse import mybir
from concourse.bass_utils import run_bass_kernel_spmd

B, V, J, H, W = 32, 4, 17, 128, 128
SIGMA = 1.0
N_CORES = 8
B_LOC = B // N_CORES          # 4 batches per core
GROUPS = B_LOC * V            # 16 (b,v) groups per core
SLICES = GROUPS * J           # 272 slices per core

# stream plan: (chunk size in slices, engine lane for the sum-of-squares).
# Bulk 17-slice chunks on ACT (1-pass square+accum), tapered tail so the
# last chunks' compute latency (900ns DMA-sem gate + engine time) doesn't
# trail the final transfer; one DVE chunk near the end unloads ACT.
PLAN = [(17, "act")] * 13 + [(13, "act")] + [
    (10, "act"), (8, "act"), (8, "act"), (5, "act"), (3, "dve"), (4, "act")
]
assert sum(n for n, _ in PLAN) == SLICES
NCK = len(PLAN)

_CACHE = {}


def _build_nc(plan=None, load_bufs=6, tail_cross=5):
    nc = bacc.Bacc()
    f32 = mybir.dt.float32
    plan = list(PLAN) if plan is None else list(plan)
    chunks = [c for c, _ in plan]
    nck = len(chunks)
    maxck = max(chunks)
    # chunks whose cross-term mul+reduce is batched into one DVE op pair
    # (their matmuls share one PSUM tile); one extra s2 output column.
    # Output columns: [0, nck) s1 per chunk; [nck, nck+tc0) s2 per non-tail
    # chunk; [nck+tc0] the batched tail s2.
    tc0 = nck - tail_cross
    tail_slices = sum(chunks[tc0:])
    NCN = nck + tc0 + 1

    pred = nc.declare_dram_parameter("pred", [SLICES, H, W], f32, isOutput=False)
    consts = nc.declare_dram_parameter(
        "consts", [2, 128 + 2 * SLICES], f32, isOutput=False
    )
    partials = nc.declare_dram_parameter(
        "partials", [1, 128, 1, NCN], f32, isOutput=True
    )

    with tile.TileContext(nc) as tc:
        with (
            tc.tile_pool(name="cpool", bufs=1) as cpool,
            tc.tile_pool(name="loads", bufs=load_bufs) as loads,
            tc.tile_pool(name="sq", bufs=2) as sqpool,
            tc.tile_pool(name="prod", bufs=2) as prodpool,
            tc.tile_pool(name="psum", bufs=4, space="PSUM") as psumpool,
            tc.tile_pool(name="gpsum", bufs=1, space="PSUM") as gpsumpool,
            tc.tile_pool(name="tailpsum", bufs=1, space="PSUM") as tailpsumpool,
            tc.tile_pool(name="outs", bufs=1) as outs,
        ):
            # ACT table warm-up (Square+Exp share a table set); the load
            # overlaps the first pred DMA
            warm = cpool.tile([128, 1], f32)
            nc.vector.memset(warm[:], 0.0)
            wsq = cpool.tile([128, 1], f32)
            nc.scalar.activation(
                out=wsq[:], in_=warm[:], func=mybir.ActivationFunctionType.Square
            )
            nc.scalar.activation(
                out=wsq[:], in_=warm[:], func=mybir.ActivationFunctionType.Exp
            )

            outcols = outs.tile([128, 1, 1, NCN], f32)

            # chunk 0 pred DMA issued FIRST so the big stream starts asap
            c0 = chunks[0]
            t0 = loads.tile([H, maxck, W], f32, tag="loads")
            dma0 = nc.sync.dma_start(
                out=t0[:, :c0, :],
                in_=pred[0:c0].rearrange("s h w -> h s w"),
            )
            dma0_name = dma0.ins.name

            # consts DMA + on-device gaussian generation:
            # gy[h,s] = exp(-0.5 (h - y_s)^2), gx likewise, via an
            # outer-product matmul (dy = h - y) then square (DVE) + exp (ACT)
            ct = cpool.tile([2, 128 + 2 * SLICES], f32)
            nc.sync.dma_start(out=ct[:], in_=consts[:, :])
            psy = gpsumpool.tile([128, SLICES], f32)
            nc.tensor.matmul(
                psy[:], ct[:, 0:128], ct[:, 128 : 128 + SLICES], start=True, stop=True
            )
            psx = gpsumpool.tile([128, SLICES], f32)
            nc.tensor.matmul(
                psx[:],
                ct[:, 0:128],
                ct[:, 128 + SLICES : 128 + 2 * SLICES],
                start=True,
                stop=True,
            )
            gsq = cpool.tile([128, 2 * SLICES], f32)
            nc.scalar.activation(
                out=gsq[:, 0:SLICES], in_=psy[:],
                func=mybir.ActivationFunctionType.Square,
            )
            nc.scalar.activation(
                out=gsq[:, SLICES:], in_=psx[:],
                func=mybir.ActivationFunctionType.Square,
            )
            gyx = cpool.tile([128, 2 * SLICES], f32)
            nc.scalar.activation(
                out=gyx[:],
                in_=gsq[:],
                func=mybir.ActivationFunctionType.Exp,
                scale=-0.5,
            )

            # ctx idxs for the partials kv-writeback; on Pool so the prep
            # (also Pool) is ordered after it without a semaphore
            idx_t = cpool.tile([128, 1], mybir.dt.int32)
            nc.gpsimd.memset(idx_t[:], 0)

            def _w(binst):
                return binst

            # main stream
            s0 = 0
            for c, (csz, lane) in enumerate(plan):
                if c == 0:
                    t = t0
                else:
                    t = loads.tile([H, maxck, W], f32, tag="loads")
                    nc.sync.dma_start(
                        out=t[:, :csz, :],
                        in_=pred[s0 : s0 + csz].rearrange("s h w -> h s w"),
                    )

                # s1 column: sum over (s, w) of pred^2, per partition h
                if lane in ("act", "x"):
                    sq = sqpool.tile([H, maxck, W], f32, tag="sq")
                    _w(nc.scalar.activation(
                        out=sq[:, :csz, :],
                        in_=t[:, :csz, :],
                        func=mybir.ActivationFunctionType.Square,
                        accum_out=outcols[:, 0, 0, c : c + 1],
                    ))
                else:
                    # square on Pool ('pd') or DVE ('dve'); reduce on DVE
                    sqeng = nc.gpsimd if lane == "pd" else nc.vector
                    sq = sqpool.tile([H, maxck, W], f32, tag="sq")
                    sqeng.tensor_mul(sq[:, :csz, :], t[:, :csz, :], t[:, :csz, :])
                    _w(nc.vector.reduce_sum(
                        outcols[:, 0, 0, c : c + 1],
                        sq[:, :csz, :],
                        axis=mybir.AxisListType.XY,
                    ))

                # s2: per-slice m' = pred_s^T @ gy_s (PSUM column)
                if c == tc0:
                    tailps = tailpsumpool.tile(
                        [128, tail_slices], f32, tag="tailps"
                    )
                    tail_s0 = s0
                if c >= tc0:
                    ps = tailps
                    poff = s0 - tail_s0
                else:
                    ps = psumpool.tile([128, maxck], f32, tag="psum")
                    poff = 0
                for sj in range(csz):
                    s = s0 + sj
                    nc.tensor.matmul(
                        ps[:, poff + sj : poff + sj + 1],
                        t[:, sj, :],
                        gyx[:, s : s + 1],
                        start=True,
                        stop=True,
                    )
                if c < tc0:
                    prod = prodpool.tile([128, maxck], f32, tag="prod")
                    nc.vector.tensor_mul(
                        prod[:, :csz],
                        ps[:, :csz],
                        gyx[:, SLICES + s0 : SLICES + s0 + csz],
                    )
                    _w(nc.vector.reduce_sum(
                        outcols[:, 0, 0, nck + c : nck + c + 1],
                        prod[:, :csz],
                        axis=mybir.AxisListType.X,
                    ))
                s0 += csz

            # batched cross for the tail chunks: one DVE mul + one reduce
            # over all their PSUM columns, into the extra s2 column
            tprod = prodpool.tile([128, tail_slices], f32, tag="tprod")
            nc.vector.tensor_mul(
                tprod[:],
                tailps[:],
                gyx[:, SLICES + tail_s0 : SLICES + tail_s0 + tail_slices],
            )
            _w(nc.vector.reduce_sum(
                outcols[:, 0, 0, nck + tc0 : nck + tc0 + 1],
                tprod[:],
                axis=mybir.AxisListType.X,
            ))

            # partials writeback: prep + trigger at the end (Tile's native
            # ordering computes sound writer waits). Post-finalize surgery
            # moves those waits from before the prep onto the trigger, so
            # the prep's ~1us SWDGE desc-gen runs early in wall-clock (Pool
            # is idle) instead of sitting on the tail critical path.
            dma_sem = nc.alloc_semaphore("pout_dma")
            prep = nc.gpsimd.kv_writeback(
                out_ap=partials[:, :, :, :],
                in_ap=outcols[:, :, :, :],
                ctx_idxs_ap=idx_t[:],
                prepare_only=True,
                sem=dma_sem,
            )
            prep_name = prep.ins.name
            trig = nc.gpsimd.trigger_dma(count=None)
            trig_name = trig.ins.name

    nc.finalize()

    fn = nc.m.functions[0]
    all_ins = [i for blk in fn.blocks for i in blk.instructions]

    # (1) Tile ticks the prep on the DMASW0 lane (the end-of-kernel drain
    # waits DMASW0 >= 16) but leaves our custom sem in on_update[0] — the
    # slot whose sem the SDMA bumps on completion. Point it at the DMASW0
    # sem so the drain accounting closes, matching the wiring a
    # non-prepared kv_writeback gets.
    dmasw = None
    for ins in all_ins:
        si = ins.sync_info
        if not si:
            continue
        for w in si.on_wait or []:
            if (w.ant_name or "").startswith("DMASW"):
                dmasw = w
                break
        if dmasw:
            break
    assert dmasw is not None, "no DMASW drain wait found"
    for ins in all_ins:
        if type(ins).__name__ == "InstKVWritebackAnt" and ins.gen_mode == 1:
            si = ins.sync_info
            upds = list(si.on_update)
            assert upds and upds[0].ant_name == "pout_dma", upds
            upds[0] = mybir.SyncUpdate(
                sync_type="semaphore",
                id=dmasw.id,
                ant_name=dmasw.ant_name,
                update_mode="sem-add-imm",
                update_value=16,
                update_reg=None,
            )
            si.on_update = upds

    # (2) relocate the writer-gating waits from before the prep onto the
    # trigger. Tile serializes Pool as [..., EventSem(DVE_tick >= M),
    # prep(wait Act_tick >= K), trigger(wait Pool_tick >= 1)]: the waits are
    # sound but force the prep's ~1us desc-gen to run after all compute.
    # Moving them to the trigger (same in-order queue, still before the DMA
    # fires) lets the prep run early; the trigger keeps the Pool-tick wait
    # so the doorbell still follows desc-gen completion.
    def _is_tick_wait(w):
        nm = w.ant_name or ""
        return (nm.startswith("Activation_") or nm.startswith("DVE_")) and \
            "sequencer" not in nm

    moved = []
    evsem = None
    for blk in fn.blocks:
        insl = blk.instructions
        trig_pos = next(
            (k for k, i in enumerate(insl) if i.name == trig_name), None
        )
        if trig_pos is None:
            continue
        prep_pos = next(k for k, i in enumerate(insl) if i.name == prep_name)
        # strip gating waits from the prep and from the contiguous
        # bookkeeping block (EventSemaphore / ring setup) just before it —
        # NOT from earlier pool compute, whose waits are load-bearing
        strip = [prep_pos]
        k = prep_pos - 1
        while k >= 0 and type(insl[k]).__name__ in (
            "InstEventSemaphore",
            "InstPseudoReloadLibraryIndex",
            "InstIncSwdgeSem",
        ):
            strip.append(k)
            k -= 1
        ev_pos = None
        for k in strip:
            ins = insl[k]
            si = ins.sync_info
            if not si or not si.on_wait:
                continue
            hit = [w for w in si.on_wait if _is_tick_wait(w)]
            if not hit:
                continue
            moved.extend(hit)
            si.on_wait = [w for w in si.on_wait if not _is_tick_wait(w)]
            if type(ins).__name__ == "InstEventSemaphore":
                evsem = ins
                ev_pos = k
        if evsem is None:
            # no EventSemaphore in the prep block carried gating waits;
            # borrow the last bookkeeping EventSemaphore before the prep
            ev_pos = next(
                k for k in strip
                if type(insl[k]).__name__ == "InstEventSemaphore"
            )
            evsem = insl[ev_pos]
        # dedupe by semaphore, keeping the strongest condition
        bysem = {}
        for w in moved:
            if w.id not in bysem or (w.wait_value or 0) > (
                bysem[w.id].wait_value or 0
            ):
                bysem[w.id] = w
        moved = list(bysem.values())
        assert 1 <= len(moved) <= 2, moved
        evsem.sync_info.on_wait = moved
        # move the gate EventSemaphore to just before the trigger so the
        # prep's desc-gen runs early while the gate still precedes the DMA
        # doorbell on the in-order Pool queue
        insl.pop(ev_pos)
        trig_pos = next(k for k, i in enumerate(insl) if i.name == trig_name)
        insl.insert(trig_pos, evsem)

    # (3) hoist the first pred DMA into the preamble block, before SP's
    # entry-barrier EventSemaphore: its SEQ decode + HWDGE descriptor-gen
    # (~650ns) then overlap the all-engine entry barrier, so the first
    # transfer starts ~640ns earlier. The DMA has no waits and its target
    # tile is untouched by the preamble.
    blk0 = fn.blocks[0]
    dma0_ins = None
    for blk in fn.blocks:
        insl = blk.instructions
        for k, ins in enumerate(insl):
            if ins.name == dma0_name:
                dma0_ins = insl.pop(k)
                break
        if dma0_ins:
            break
    assert dma0_ins is not None
    blk0.instructions.insert(1, dma0_ins)

    # (4) the six exit drain EventSemaphores on SP serialize at ~50ns each;
    # put the DMASW0 (partials writeback) wait on the LAST of them so the
    # others complete during the stream instead of after the kv sem fires.
    exit_blk = fn.blocks[-1]
    drains = [
        i for i in exit_blk.instructions
        if type(i).__name__ == "InstEventSemaphore"
        and str(i.engine).endswith("SP")
        and i.sync_info and len(i.sync_info.on_wait or []) == 2
    ]
    if len(drains) >= 2:
        src = next(
            (d for d in drains
             if any((w.ant_name or "").startswith("DMASW")
                    for w in d.sync_info.on_wait)),
            None,
        )
        dst = drains[-1]
        if src is not None and src is not dst:
            sw = src.sync_info.on_wait
            dw = dst.sync_info.on_wait
            si = next(k for k, w in enumerate(sw)
                      if (w.ant_name or "").startswith("DMASW"))
            sw[si], dw[-1] = dw[-1], sw[si]
            src.sync_info.on_wait = sw
            dst.sync_info.on_wait = dw
    return nc


def _joints_2d(proj_mats_batch, joints_3d_gt_batch):
    """Projected 2D joints [B, V, J, 2] (x, y) in float32 (reference math)."""
    joints = joints_3d_gt_batch.astype(np.float32)
    ones = np.ones(joints.shape[:-1] + (1,), dtype=np.float32)
    joints_h = np.concatenate([joints, ones], axis=-1)  # [B, J, 4]
    proj = np.einsum(
        "bvcd,bjd->bvjc", proj_mats_batch.astype(np.float32), joints_h
    ).astype(np.float32)  # [B, V, J, 3]
    return proj[..., :2] / proj[..., 2:3]


def kernel(heatmaps_pred, proj_mats_batch, joints_3d_gt_batch, joints_3d_valid_batch,
           _profile=None):
    heatmaps_pred = np.ascontiguousarray(np.asarray(heatmaps_pred, dtype=np.float32))
    joints_2d = _joints_2d(np.asarray(proj_mats_batch), np.asarray(joints_3d_gt_batch))

    # s3 = sum over slices of (sum_h gy^2) * (sum_w gx^2), exact in f64
    xs = np.arange(W, dtype=np.float64)
    ys = np.arange(H, dtype=np.float64)
    dx2 = (xs - joints_2d[..., 0, None].astype(np.float64)) ** 2  # [B,V,J,W]
    dy2 = (ys - joints_2d[..., 1, None].astype(np.float64)) ** 2  # [B,V,J,H]
    gx2 = np.exp(-dx2)  # gx^2 = exp(-dx2) for sigma=1
    gy2 = np.exp(-dy2)
    s3 = float((gy2.sum(-1) * gx2.sum(-1)).sum())

    if "nc" not in _CACHE:
        _CACHE["nc"] = _build_nc()
    nc = _CACHE["nc"]

    hramp = np.arange(128, dtype=np.float32)
    in_maps = []
    for c in range(N_CORES):
        bsl = slice(B_LOC * c, B_LOC * (c + 1))
        # slice order: (b_local, v, j) -> s
        y = joints_2d[bsl, :, :, 1].reshape(SLICES).astype(np.float32)
        x = joints_2d[bsl, :, :, 0].reshape(SLICES).astype(np.float32)
        consts = np.empty((2, 128 + 2 * SLICES), dtype=np.float32)
        consts[0, 0:128] = hramp
        consts[1, 0:128] = 1.0
        consts[0, 128:] = 1.0
        consts[1, 128 : 128 + SLICES] = -y
        consts[1, 128 + SLICES :] = -x
        in_maps.append(
            {
                "pred": heatmaps_pred[bsl].reshape(SLICES, H, W),
                "consts": consts,
            }
        )

    res = run_bass_kernel_spmd(nc, in_maps, core_ids=list(range(N_CORES)))
    if _profile is not None:
        _profile["result"] = res
        _profile["in_maps"] = in_maps

    s1 = 0.0
    s2 = 0.0
    for c in range(N_CORES):
        p = res.results[c]["partials"].astype(np.float64)  # [1, 128, 1, 2*NCK]
        s1 += p[0, :, 0, :NCK].sum()
        s2 += p[0, :, 0, NCK:].sum()

    total = s1 - 2.0 * s2 + s3
    return np.float32(total / (B * V * J * H * W))
